# revision 18
# baseline (speedup 1.0000x reference)
"""EngineeringGNN Trainium2 kernel: 8-core SPMD GINE message passing.

Sharding: nodes padded to 100352 and split contiguously over 8 cores
(12544 each). Edges assigned to their destination node's owner, grouped by
128-node destination windows so segment-sum becomes one-hot matmuls
accumulating in PSUM. h[src] rows come from a replicated (AllGathered)
token-major table via indirect DMA gather. Small weights replicated.
"""
import os
import sys
import numpy as np
import ml_dtypes

sys.path.insert(0, "/opt/trn_rl_repo")

import concourse.bass as bass
import concourse.bacc as bacc
import concourse.tile as tile
from concourse import mybir
from concourse.bass_utils import run_bass_kernel_spmd
from concourse.masks import make_identity

F32 = mybir.dt.float32
BF16 = mybir.dt.bfloat16
I32 = mybir.dt.int32
AX = mybir.AxisListType.X
OP = mybir.AluOpType
AF = mybir.ActivationFunctionType

P = 128
H = 128
NCORES = 8
N_REAL = 100000
NPAD = 100352            # 8 * 12544
NC_NODES = NPAD // NCORES  # 12544
NW = NC_NODES // P         # 98 windows per core
B = 16
ND, ED, L = 12, 6, 3
MIN_SCALE = 1e-5
YIELD = 2.5e8
LN_EPS = 1e-5
GG = 8                   # edge tiles per gather group
WBATCH = 8              # windows per node-phase batch (98 = 7*14)
BIG = 1e30


def _prep(inputs):
    """Host-side sharding prep. Returns (schedule, in_maps)."""
    x = np.asarray(inputs["x"], np.float32)
    ea = np.asarray(inputs["edge_attr"], np.float32)
    ei = np.asarray(inputs["edge_index"]).astype(np.int64)
    batch = np.asarray(inputs["batch"]).astype(np.int64)
    pos = np.asarray(inputs["pos"], np.float32)
    src_g, dst_g = ei[0], ei[1]

    owner = dst_g // NC_NODES
    win_g = (dst_g % NC_NODES) // P

    counts = np.zeros((NCORES, NW), np.int64)
    np.add.at(counts, (owner, win_g), 1)
    tiles_per_w = np.maximum(1, np.ceil(counts / P).astype(np.int64)).max(axis=0)
    T = int(tiles_per_w.sum())

    sched_w = np.empty(T, np.int32)
    sched_start = np.zeros(T, bool)
    sched_stop = np.zeros(T, bool)
    t0 = 0
    for w in range(NW):
        tw = int(tiles_per_w[w])
        sched_w[t0:t0 + tw] = w
        sched_start[t0] = True
        sched_stop[t0 + tw - 1] = True
        t0 += tw
    w_tile0 = np.concatenate([[0], np.cumsum(tiles_per_w)[:-1]])

    order = np.lexsort((win_g, owner))
    so, wo = src_g[order], win_g[order]
    do = dst_g[order] % P
    eao = ea[order]
    owner_sorted = owner[order]

    in_maps = []
    z = pos[:, 2]
    for c in range(NCORES):
        lo = np.searchsorted(owner_sorted, c)
        hi = np.searchsorted(owner_sorted, c + 1)
        sc, wc, dc, eac = so[lo:hi], wo[lo:hi], do[lo:hi], eao[lo:hi]
        src_t = np.zeros((P, T), np.int32)
        dst_t = np.full((P, T), -1.0, np.float32)
        ea_t = np.zeros((P, T, ED), np.float32)
        wstart = np.searchsorted(wc, np.arange(NW + 1))
        for w in range(NW):
            a, b = int(wstart[w]), int(wstart[w + 1])
            n = b - a
            t_base = int(w_tile0[w])
            nt = int(np.ceil(n / P)) if n else 0
            for k in range(nt):
                s = a + k * P
                e = min(a + (k + 1) * P, b)
                m = e - s
                src_t[:m, t_base + k] = sc[s:e]
                dst_t[:m, t_base + k] = dc[s:e].astype(np.float32)
                ea_t[:m, t_base + k] = eac[s:e]
        ea_fm = ea_t.transpose(2, 1, 0).reshape(ED, T * P).copy()

        n0 = c * NC_NODES
        n_real = max(0, min(NC_NODES, N_REAL - n0))
        xc = np.zeros((NC_NODES, ND), np.float32)
        xc[:n_real] = x[n0:n0 + n_real]
        bc = np.full(NC_NODES, B, np.int64)
        bc[:n_real] = batch[n0:n0 + n_real]
        zc = np.zeros(NC_NODES, np.float32)
        zc[:n_real] = z[n0:n0 + n_real]

        x_fm = xc.T.copy()
        batch_t = bc.reshape(NW, P).T.astype(np.float32).copy()
        z_t = zc.reshape(NW, P).T.copy()
        boh = np.zeros((B, NC_NODES), np.float32)
        valid = bc < B
        boh[bc[valid], np.nonzero(valid)[0]] = 1.0

        in_maps.append({
            "x_fm": x_fm.astype(ml_dtypes.bfloat16),
            "ea_fm": ea_fm.astype(ml_dtypes.bfloat16),
            "src_i": src_t, "dstrel": dst_t,
            "batch_t": batch_t, "z_t": z_t, "bonehot": boh,
        })

    rep = {
        "fv": np.asarray(inputs["force_vector"], np.float32),
        "mp": np.asarray(inputs["material_params"], np.float32),
        "log_base": np.asarray(inputs["log_base"], np.float32).reshape(1, 1),
    }
    for nm in ["dh_w1", "dh_w2", "sm_w1", "sm_w2", "sh_w1", "sh_w2"]:
        rep[nm] = np.asarray(inputs[nm], np.float32)
    for nm in ["ne_w1", "ne_w2", "ee_w1", "ee_w2"]:
        rep[nm] = np.asarray(inputs[nm], np.float32).astype(ml_dtypes.bfloat16)
    for nm in ["ne_b1", "ne_b2", "ee_b1", "ee_b2", "dh_b1", "dh_b2",
               "sm_b1", "sm_b2", "sh_b1", "sh_b2"]:
        rep[nm] = np.asarray(inputs[nm], np.float32).reshape(-1, 1)
    for nm, key in [("ne_gr", "ne_g"), ("ne_br", "ne_be"),
                    ("ee_gr", "ee_g"), ("ee_br", "ee_be")]:
        rep[nm] = np.tile(np.asarray(inputs[key], np.float32)[None, :], (P, 1))
    cw1 = np.asarray(inputs["conv_w1"], np.float32)
    cw2 = np.asarray(inputs["conv_w2"], np.float32)
    cb1 = np.asarray(inputs["conv_b1"], np.float32)
    cb2 = np.asarray(inputs["conv_b2"], np.float32)
    png = np.asarray(inputs["pn_g"], np.float32)
    pnb = np.asarray(inputs["pn_b"], np.float32)
    for i in range(L):
        rep[f"cw1_{i}"] = cw1[i].astype(ml_dtypes.bfloat16)
        rep[f"cw2_{i}"] = cw2[i].astype(ml_dtypes.bfloat16)
        rep[f"cb1_{i}"] = cb1[i].reshape(-1, 1)
        rep[f"cb2_{i}"] = cb2[i].reshape(-1, 1)
        rep[f"pn_gr_{i}"] = np.tile(png[i][None, :], (P, 1))
        rep[f"pn_br_{i}"] = np.tile(pnb[i][None, :], (P, 1))
    for m in in_maps:
        m.update(rep)
    sched = (T, sched_w, sched_start, sched_stop)
    return sched, in_maps


def _ln_tokmajor(nc, pool, y, g_rep, b_rep, out_ap, eps_ap=None):
    """LayerNorm over features on token-major y [128, 128] -> out_ap."""
    stats = pool.tile([P, 6], F32, tag="ln_stats", name="ln_stats")
    nc.vector.bn_stats(out=stats[:], in_=y[:])
    mv = pool.tile([P, 2], F32, tag="ln_mv", name="ln_mv")
    nc.vector.bn_aggr(out=mv[:], in_=stats[:])
    rstd = pool.tile([P, 1], F32, tag="ln_rstd", name="ln_rstd")
    nc.scalar.activation(out=rstd[:], in_=mv[:, 1:2], func=AF.Sqrt, bias=eps_ap)
    nc.vector.reciprocal(out=rstd[:], in_=rstd[:])
    yn = pool.tile([P, H], F32, tag="ln_yn", name="ln_yn")
    nc.vector.tensor_scalar(out=yn[:], in0=y[:], scalar1=mv[:, 0:1],
                            scalar2=rstd[:], op0=OP.subtract, op1=OP.mult)
    nc.vector.tensor_tensor(out=yn[:], in0=yn[:], in1=g_rep[:], op=OP.mult)
    nc.vector.tensor_tensor(out=out_ap, in0=yn[:], in1=b_rep[:], op=OP.add)


def build(sched):
    T, sched_w, sched_start, sched_stop = sched
    nc = bacc.Bacc("TRN2", target_bir_lowering=False, debug=False,
                   num_devices=NCORES)

    def din(name, shape, dt=F32):
        return nc.dram_tensor(name, list(shape), dt, kind="ExternalInput")

    x_fm = din("x_fm", [ND, NC_NODES], BF16)
    ea_fm = din("ea_fm", [ED, T * P], BF16)
    src_i = din("src_i", [P, T], I32)
    dstrel = din("dstrel", [P, T])
    batch_t = din("batch_t", [P, NW])
    z_t = din("z_t", [P, NW])
    bonehot = din("bonehot", [B, NC_NODES])
    fv = din("fv", [B, 3])
    mp = din("mp", [B, 2])
    log_base = din("log_base", [1, 1])
    wts = {}
    for nm, shp in [("dh_w1", (H, 64)), ("dh_w2", (64, 3)),
                    ("sm_w1", (4, 64)), ("sm_w2", (64, 1)),
                    ("sh_w1", (H, 64)), ("sh_w2", (64, 1)),
                    ("ne_b1", (H, 1)), ("ne_b2", (H, 1)),
                    ("ee_b1", (H, 1)), ("ee_b2", (H, 1)),
                    ("dh_b1", (64, 1)), ("dh_b2", (3, 1)),
                    ("sm_b1", (64, 1)), ("sm_b2", (1, 1)),
                    ("sh_b1", (64, 1)), ("sh_b2", (1, 1)),
                    ("ne_gr", (P, H)), ("ne_br", (P, H)),
                    ("ee_gr", (P, H)), ("ee_br", (P, H))]:
        wts[nm] = din(nm, shp)
    bf_wts = {}
    for nm, shp in [("ne_w1", (ND, H)), ("ne_w2", (H, H)),
                    ("ee_w1", (ED, H)), ("ee_w2", (H, H))]:
        bf_wts[nm] = nc.dram_tensor(nm, list(shp), BF16, kind="ExternalInput")
    for i in range(L):
        for nm, shp in [(f"cb1_{i}", (H, 1)), (f"cb2_{i}", (H, 1)),
                        (f"pn_gr_{i}", (P, H)), (f"pn_br_{i}", (P, H))]:
            wts[nm] = din(nm, shp)
        for nm, shp in [(f"cw1_{i}", (H, H)), (f"cw2_{i}", (H, H))]:
            bf_wts[nm] = nc.dram_tensor(nm, list(shp), BF16, kind="ExternalInput")

    def dout(name, shape):
        return nc.dram_tensor(name, list(shape), F32, kind="ExternalOutput")

    o_raw_u = dout("o_raw_u", [3, NC_NODES])
    o_u = dout("o_u", [3, NC_NODES])
    o_log_s = dout("o_log_s", [1, NC_NODES])
    o_s = dout("o_s", [1, NC_NODES])
    o_safety = dout("o_safety", [1, NC_NODES])
    o_dsg = dout("o_dsg", [1, B])
    o_dsg_mean = dout("o_dsg_mean", [1, 1])

    e_perm = nc.dram_tensor("e_perm", [P, T * H], BF16)
    h_next = [nc.dram_tensor(f"h_next_{i}", [NC_NODES, H], BF16) for i in range(L)]
    h_tbl = [nc.dram_tensor(f"h_tbl_{i}", [NPAD, H], BF16, addr_space="Shared")
             for i in range(L)]
    zst_in = nc.dram_tensor("zst_in", [64, 1], F32)
    zst_out = nc.dram_tensor("zst_out", [64, 1], F32, addr_space="Shared")

    RG = [list(range(NCORES))]

    with tile.TileContext(nc) as tc:
        import contextlib
        with contextlib.ExitStack() as ctx:
            cstp = ctx.enter_context(tc.tile_pool(name="consts", bufs=1))
            hop = ctx.enter_context(tc.tile_pool(name="hown", bufs=1))
            agp = ctx.enter_context(tc.tile_pool(name="aggrp", bufs=1))
            sb = ctx.enter_context(tc.tile_pool(name="work", bufs=2))
            nb = ctx.enter_context(tc.tile_pool(name="nodework", bufs=2))
            lnp = ctx.enter_context(tc.tile_pool(name="lnwork", bufs=2))
            ps = ctx.enter_context(tc.tile_pool(name="ps", bufs=1, space="PSUM"))

            # ---------- constants ----------
            ident = cstp.tile([P, P], F32)
            make_identity(nc, ident[:])
            ident_bf = cstp.tile([P, P], BF16)
            nc.vector.tensor_copy(out=ident_bf[:], in_=ident[:])
            ceps = cstp.tile([P, 1], F32)
            nc.vector.memset(ceps[:], LN_EPS)
            c12 = cstp.tile([B, 1], F32)
            nc.vector.memset(c12[:], 1e-12)
            c6 = cstp.tile([B, 1], F32)
            nc.vector.memset(c6[:], 1e-6)
            iota_f = cstp.tile([P, GG * P], F32)
            nc.gpsimd.iota(iota_f[:], pattern=[[0, GG], [1, P]], base=0,
                           channel_multiplier=0,
                           allow_small_or_imprecise_dtypes=True)
            srcs = cstp.tile([P, T], I32)
            nc.sync.dma_start(out=srcs[:], in_=src_i[:, :])
            dsts = cstp.tile([P, T], F32)
            nc.sync.dma_start(out=dsts[:], in_=dstrel[:, :])
            W = {}
            for nm, hdl in wts.items():
                shp = list(hdl.shape)
                tl = cstp.tile(shp, F32, name=f"w_{nm}")
                nc.sync.dma_start(out=tl[:], in_=hdl[:, :])
                W[nm] = tl
            for nm, hdl in bf_wts.items():
                shp = list(hdl.shape)
                tl = cstp.tile(shp, BF16, name=f"wb_{nm}")
                nc.sync.dma_start(out=tl[:], in_=hdl[:, :])
                W[nm] = tl

            h_own = hop.tile([P, NW, H], F32)
            aggr_sb = agp.tile([P, NW, H], F32)

            def fm_mlp_to(fm_out_cb, rx_hdl, rx_p, w1, b1, w2, b2, g_rep, b_rep,
                          n_tiles, tagpfx):
                """Encoder: feature-major 2-layer MLP + transpose + LN.
                fm_out_cb(j_tile, token_major_ln_out_writer)"""
                for g0 in range(0, n_tiles, 4):
                    gn = min(4, n_tiles - g0)
                    cols = gn * P
                    rx = nb.tile([rx_p, 4 * P], BF16, tag="rx", name="rx")
                    nc.sync.dma_start(out=rx[:, :cols],
                                      in_=rx_hdl[:, g0 * P:g0 * P + cols])
                    p1 = ps.tile([P, 512], F32, space="PSUM", tag="pb", name="p1")
                    nc.tensor.matmul(p1[:, :cols], lhsT=w1[:], rhs=rx[:, :cols],
                                     start=True, stop=True)
                    t1 = nb.tile([P, 4 * P], BF16, tag="t512a1", name="t1")
                    nc.scalar.activation(out=t1[:, :cols], in_=p1[:, :cols],
                                         func=AF.Relu, bias=b1[:])
                    p2 = ps.tile([P, 512], F32, space="PSUM", tag="pc", name="p2")
                    nc.tensor.matmul(p2[:, :cols], lhsT=w2[:], rhs=t1[:, :cols],
                                     start=True, stop=True)
                    t2 = nb.tile([P, 4 * P], F32, tag="t512b", name="t2")
                    nc.vector.tensor_scalar(out=t2[:, :cols], in0=p2[:, :cols],
                                            scalar1=b2[:], scalar2=None,
                                            op0=OP.add)
                    for j in range(gn):
                        pt = ps.tile([P, P], F32, space="PSUM", tag="pa",
                                     name="ptr", bufs=2)
                        nc.tensor.transpose(out=pt[:],
                                            in_=t2[:, j * P:(j + 1) * P],
                                            identity=ident[:])
                        yj = nb.tile([P, P], F32, tag="t128a", name="yj")
                        nc.scalar.copy(out=yj[:], in_=pt[:])
                        fm_out_cb(g0 + j, yj, g_rep, b_rep)

            # ---------- node encoder ----------
            def ne_out(w, yj, g_rep, b_rep):
                _ln_tokmajor(nc, lnp, yj, g_rep, b_rep, h_own[:, w, :], ceps[:])
                hb = nb.tile([P, P], BF16, tag="hb16", name="hb16")
                nc.vector.tensor_copy(out=hb[:], in_=h_own[:, w, :])
                nc.sync.dma_start(out=h_next[0][w * P:(w + 1) * P, :], in_=hb[:])
            fm_mlp_to(ne_out, x_fm, ND, W["ne_w1"], W["ne_b1"], W["ne_w2"],
                      W["ne_b2"], W["ne_gr"], W["ne_br"], NW, "ne")

            # ---------- edge encoder ----------
            def ee_out(t, yj, g_rep, b_rep):
                ej = nb.tile([P, P], BF16, tag="t128b", name="ej")
                _ln_tokmajor(nc, lnp, yj, g_rep, b_rep, ej[:], ceps[:])
                nc.sync.dma_start(out=e_perm[:, t * H:(t + 1) * H], in_=ej[:])
            fm_mlp_to(ee_out, ea_fm, ED, W["ee_w1"], W["ee_b1"], W["ee_w2"],
                      W["ee_b2"], W["ee_gr"], W["ee_br"], T, "ee")

            # ---------- edge encoder ----------
            def ee_out(t, yj, g_rep, b_rep):
                ej = nb.tile([P, P], BF16, tag="t128b", name="ej")
                _ln_tokmajor(nc, lnp, yj, g_rep, b_rep, ej[:], ceps[:])
                nc.sync.dma_start(out=e_perm[:, t * H:(t + 1) * H], in_=ej[:])
            fm_mlp_to(ee_out, ea_fm, ED, W["ee_w1"], W["ee_b1"], W["ee_w2"],
                      W["ee_b2"], W["ee_gr"], W["ee_br"], T, "ee")

            nc.gpsimd.collective_compute(
                "AllGather", OP.bypass, replica_groups=RG,
                ins=[h_next[0][:, :]], outs=[h_tbl[0][:, :]])

            # ---------- GINE layers ----------
            for li in range(L):
                tbl = h_tbl[li]
                pseg = {}
                for g0 in range(0, T, GG):
                    gn = min(GG, T - g0)
                    cols = gn * P
                    hg = sb.tile([P, GG * P], BF16, tag="hg", name="hg")
                    for k in range(gn):
                        t = g0 + k
                        nc.gpsimd.indirect_dma_start(
                            out=hg[:, k * H:(k + 1) * H], out_offset=None,
                            in_=tbl[:, :],
                            in_offset=bass.IndirectOffsetOnAxis(
                                ap=srcs[:, t:t + 1], axis=0))
                    eg = sb.tile([P, GG * P], BF16, tag="eg", name="eg")
                    nc.sync.dma_start(out=eg[:, :cols],
                                      in_=e_perm[:, g0 * H:g0 * H + cols])
                    mg = sb.tile([P, GG * P], BF16, tag="mg", name="mg")
                    nc.vector.tensor_add(out=mg[:, :cols], in0=hg[:, :cols],
                                         in1=eg[:, :cols])
                    nc.scalar.activation(out=mg[:, :cols], in_=mg[:, :cols],
                                         func=AF.Relu)
                    sg = sb.tile([P, GG * P], BF16, tag="sg", name="sg")
                    d_ap = bass.AP(tensor=dsts.tensor,
                                   offset=dsts[:, g0:g0 + gn].offset,
                                   ap=[dsts.ap[0], [dsts.ap[1][0], gn], [0, P]])
                    nc.vector.tensor_tensor(out=sg[:, :cols], in0=d_ap,
                                            in1=iota_f[:, :cols], op=OP.is_equal)
                    for k in range(gn):
                        t = g0 + k
                        w = int(sched_w[t])
                        if bool(sched_start[t]):
                            pseg[w] = ps.tile([P, H], F32, space="PSUM",
                                              tag=f"pseg{w % 4}", name=f"psg{w % 4}")
                        nc.tensor.matmul(pseg[w][:],
                                         lhsT=sg[:, k * P:(k + 1) * P],
                                         rhs=mg[:, k * H:(k + 1) * H],
                                         start=bool(sched_start[t]),
                                         stop=bool(sched_stop[t]))
                        if bool(sched_stop[t]):
                            nc.vector.tensor_copy(out=aggr_sb[:, w, :],
                                                  in_=pseg[w][:])
                            del pseg[w]

                for b0 in range(0, NW, WBATCH):
                    bn_ = min(WBATCH, NW - b0)
                    fmb = nb.tile([P, WBATCH * P], BF16, tag="fmb", name="fmb",
                                  bufs=1)
                    for j in range(bn_):
                        w = b0 + j
                        ain = nb.tile([P, P], F32, tag="t128a", name="ain")
                        nc.vector.tensor_add(out=ain[:], in0=h_own[:, w, :],
                                             in1=aggr_sb[:, w, :])
                        pt = ps.tile([P, P], F32, space="PSUM", tag="pa",
                                     name="np_pt", bufs=2)
                        nc.tensor.transpose(out=pt[:], in_=ain[:],
                                            identity=ident[:])
                        nc.scalar.copy(out=fmb[:, j * P:(j + 1) * P], in_=pt[:])
                    for s0 in range(0, bn_ * P, 512):
                        sc = min(512, bn_ * P - s0)
                        p1 = ps.tile([P, 512], F32, space="PSUM", tag="pb",
                                     name="np_p1")
                        nc.tensor.matmul(p1[:, :sc], lhsT=W[f"cw1_{li}"][:],
                                         rhs=fmb[:, s0:s0 + sc], start=True,
                                         stop=True)
                        tm = nb.tile([P, 512], BF16, tag="t512a1", name="np_tm")
                        nc.scalar.activation(out=tm[:, :sc], in_=p1[:, :sc],
                                             func=AF.Relu, bias=W[f"cb1_{li}"][:])
                        p2 = ps.tile([P, 512], F32, space="PSUM", tag="pc",
                                     name="np_p2")
                        nc.tensor.matmul(p2[:, :sc], lhsT=W[f"cw2_{li}"][:],
                                         rhs=tm[:, :sc], start=True, stop=True)
                        nc.scalar.activation(out=fmb[:, s0:s0 + sc],
                                             in_=p2[:, :sc], func=AF.Relu,
                                             bias=W[f"cb2_{li}"][:])
                    for j in range(bn_):
                        w = b0 + j
                        pt = ps.tile([P, P], BF16, space="PSUM", tag="pa",
                                     name="np_pt2", bufs=2)
                        nc.tensor.transpose(out=pt[:], in_=fmb[:, j * P:(j + 1) * P],
                                            identity=ident_bf[:])
                        yj = nb.tile([P, P], F32, tag="t128a", name="np_yj")
                        nc.vector.tensor_add(out=yj[:], in0=h_own[:, w, :],
                                             in1=pt[:])
                        _ln_tokmajor(nc, lnp, yj, W[f"pn_gr_{li}"],
                                     W[f"pn_br_{li}"], h_own[:, w, :], ceps[:])
                        if li < L - 1:
                            hb = nb.tile([P, P], BF16, tag="hb16", name="hb16")
                            nc.vector.tensor_copy(out=hb[:], in_=h_own[:, w, :])
                            nc.sync.dma_start(
                                out=h_next[li + 1][w * P:(w + 1) * P, :],
                                in_=hb[:])
                if li < L - 1:
                    nc.gpsimd.collective_compute(
                        "AllGather", OP.bypass, replica_groups=RG,
                        ins=[h_next[li + 1][:, :]], outs=[h_tbl[li + 1][:, :]])

            # ---------- per-graph scale path (independent of h) ----------
            bt_sb = cstp.tile([P, NW], F32)
            nc.sync.dma_start(out=bt_sb[:], in_=batch_t[:, :])
            zt_sb = cstp.tile([P, NW], F32)
            nc.sync.dma_start(out=zt_sb[:], in_=z_t[:, :])
            nz_sb = cstp.tile([P, NW], F32)
            nc.vector.tensor_scalar(out=nz_sb[:], in0=zt_sb[:], scalar1=-1.0,
                                    scalar2=None, op0=OP.mult)
            zred = cstp.tile([P, 64], F32)
            for b in range(B):
                msk = lnp.tile([P, NW], mybir.dt.uint8, tag="zmask", name="zmask")
                nc.vector.tensor_scalar(out=msk[:], in0=bt_sb[:],
                                        scalar1=float(b), scalar2=None,
                                        op0=OP.is_equal)
                zm = lnp.tile([P, NW], F32, tag="zm", name="zm")
                nc.vector.memset(zm[:], -BIG)
                nc.vector.copy_predicated(zm[:], msk[:], zt_sb[:])
                nc.vector.tensor_reduce(out=zred[:, b:b + 1], in_=zm[:],
                                        axis=AX, op=OP.max)
                zm2 = lnp.tile([P, NW], F32, tag="zm", name="zm2")
                nc.vector.memset(zm2[:], -BIG)
                nc.vector.copy_predicated(zm2[:], msk[:], nz_sb[:])
                nc.vector.tensor_reduce(out=zred[:, 32 + b:33 + b], in_=zm2[:],
                                        axis=AX, op=OP.max)
            pzt = ps.tile([64, P], F32, space="PSUM", tag="pa", name="pzt",
                          bufs=2)
            nc.tensor.transpose(out=pzt[:], in_=zred[:], identity=ident[:])
            zcols = cstp.tile([64, P], F32)
            nc.scalar.copy(out=zcols[:], in_=pzt[:])
            zst = cstp.tile([64, 1], F32)
            nc.vector.tensor_reduce(out=zst[:], in_=zcols[:], axis=AX, op=OP.max)
            nc.sync.dma_start(out=zst_in[:, :], in_=zst[:])
            nc.gpsimd.collective_compute(
                "AllReduce", OP.max, replica_groups=RG,
                ins=[zst_in[:, :]], outs=[zst_out[:, :]])
            zmax_t = cstp.tile([B, 1], F32)
            nc.sync.dma_start(out=zmax_t[:], in_=zst_out[0:B, :])
            zmin_t = cstp.tile([B, 1], F32)
            nc.sync.dma_start(out=zmin_t[:], in_=zst_out[32:32 + B, :])
            geom = cstp.tile([B, 1], F32)
            nc.vector.tensor_tensor(out=geom[:], in0=zmax_t[:],
                                    in1=zmin_t[:], op=OP.add)
            nc.vector.tensor_scalar(out=geom[:], in0=geom[:], scalar1=1e-6,
                                    scalar2=None, op0=OP.max)
            feats = cstp.tile([B, 4], F32)
            fv_sb = cstp.tile([B, 3], F32)
            nc.sync.dma_start(out=fv_sb[:], in_=fv[:, :])
            mp_sb = cstp.tile([B, 2], F32)
            nc.sync.dma_start(out=mp_sb[:], in_=mp[:, :])
            fsq = cstp.tile([B, 3], F32)
            nc.vector.tensor_tensor(out=fsq[:], in0=fv_sb[:], in1=fv_sb[:],
                                    op=OP.mult)
            fm_ = cstp.tile([B, 1], F32)
            nc.vector.tensor_reduce(out=fm_[:], in_=fsq[:], axis=AX, op=OP.add)
            nc.scalar.activation(out=fm_[:], in_=fm_[:], func=AF.Sqrt)
            nc.scalar.activation(out=feats[:, 0:1], in_=fm_[:], func=AF.Ln,
                                 bias=1.0)
            em_ = cstp.tile([B, 1], F32)
            nc.vector.tensor_scalar(out=em_[:], in0=mp_sb[:, 0:1], scalar1=1.0,
                                    scalar2=None, op0=OP.max)
            nc.scalar.activation(out=feats[:, 1:2], in_=em_[:], func=AF.Ln,
                                 bias=c12[:])
            nc.vector.tensor_scalar(out=feats[:, 2:3], in0=mp_sb[:, 1:2],
                                    scalar1=0.0, scalar2=0.49, op0=OP.max,
                                    op1=OP.min)
            nc.scalar.activation(out=feats[:, 3:4], in_=geom[:], func=AF.Ln,
                                 bias=c6[:])
            pft = ps.tile([4, B], F32, space="PSUM", tag="pb", name="pft")
            nc.tensor.transpose(out=pft[:], in_=feats[:],
                                identity=ident[0:B, 0:B])
            ftm = cstp.tile([4, B], F32)
            nc.scalar.copy(out=ftm[:], in_=pft[:])
            psm1 = ps.tile([64, B], F32, space="PSUM", tag="pc", name="psm1")
            nc.tensor.matmul(psm1[:], lhsT=W["sm_w1"][:], rhs=ftm[:], start=True,
                             stop=True)
            tsm = cstp.tile([64, B], F32)
            nc.scalar.activation(out=tsm[:], in_=psm1[:], func=AF.Relu,
                                 bias=W["sm_b1"][:])
            psm2 = ps.tile([1, B], F32, space="PSUM", tag="pseg2", name="psm2")
            nc.tensor.matmul(psm2[:], lhsT=W["sm_w2"][:], rhs=tsm[:], start=True,
                             stop=True)
            lmu = cstp.tile([1, B], F32)
            nc.vector.tensor_scalar(out=lmu[:], in0=psm2[:],
                                    scalar1=W["sm_b2"][:], scalar2=None,
                                    op0=OP.add)
            nc.vector.tensor_scalar(out=lmu[:], in0=lmu[:], scalar1=-8.0,
                                    scalar2=8.0, op0=OP.max, op1=OP.min)
            nc.scalar.activation(out=lmu[:], in_=lmu[:], func=AF.Exp)
            lb_sb = cstp.tile([1, 1], F32)
            nc.sync.dma_start(out=lb_sb[:], in_=log_base[:, :])
            base_sb = cstp.tile([1, 1], F32)
            nc.scalar.activation(out=base_sb[:], in_=lb_sb[:], func=AF.Exp)
            nc.scalar.activation(out=base_sb[:], in_=base_sb[:], func=AF.Ln,
                                 bias=1.0)
            nc.vector.tensor_scalar(out=base_sb[:], in0=base_sb[:],
                                    scalar1=MIN_SCALE, scalar2=None, op0=OP.add)
            dsg_sb = cstp.tile([1, B], F32)
            nc.vector.tensor_scalar(out=dsg_sb[:], in0=lmu[:], scalar1=base_sb[:],
                                    scalar2=MIN_SCALE, op0=OP.mult, op1=OP.max)
            nc.sync.dma_start(out=o_dsg[:, :], in_=dsg_sb[:])
            dmean = cstp.tile([1, 1], F32)
            nc.vector.tensor_reduce(out=dmean[:], in_=dsg_sb[:], axis=AX,
                                    op=OP.add)
            nc.vector.tensor_scalar(out=dmean[:], in0=dmean[:],
                                    scalar1=1.0 / B, scalar2=None, op0=OP.mult)
            nc.sync.dma_start(out=o_dsg_mean[:, :], in_=dmean[:])
            pdt = ps.tile([B, 1], F32, space="PSUM", tag="pseg3", name="pdt")
            nc.tensor.transpose(out=pdt[:], in_=dsg_sb[:],
                                identity=ident[0:1, 0:1])
            dsg_t = cstp.tile([B, 1], F32)
            nc.scalar.copy(out=dsg_t[:], in_=pdt[:])
            dsg3 = cstp.tile([B, 3], F32)
            nc.vector.tensor_copy(out=dsg3[:], in_=dsg_t[:].to_broadcast([B, 3]))

            # ---------- heads (streamed per 4-window group) ----------
            for b0 in range(0, NW, 4):
                bn_ = min(4, NW - b0)
                cols = bn_ * P
                fmb = nb.tile([P, 512], F32, tag="t512b", name="hd_fmb")
                for j in range(bn_):
                    pt = ps.tile([P, P], F32, space="PSUM", tag="pa",
                                 name="hd_pt", bufs=2)
                    nc.tensor.transpose(out=pt[:], in_=h_own[:, b0 + j, :],
                                        identity=ident[:])
                    nc.scalar.copy(out=fmb[:, j * P:(j + 1) * P], in_=pt[:])
                # displacement head
                p1 = ps.tile([64, 512], F32, space="PSUM", tag="pb", name="hd_p1")
                nc.tensor.matmul(p1[:, :cols], lhsT=W["dh_w1"][:],
                                 rhs=fmb[:, :cols], start=True, stop=True)
                td = nb.tile([64, 512], F32, tag="t512a", name="hd_td")
                nc.scalar.activation(out=td[:, :cols], in_=p1[:, :cols],
                                     func=AF.Relu, bias=W["dh_b1"][:])
                p2 = ps.tile([3, 512], F32, space="PSUM", tag="pc", name="hd_p2")
                nc.tensor.matmul(p2[:, :cols], lhsT=W["dh_w2"][:], rhs=td[:, :cols],
                                 start=True, stop=True)
                ruj = nb.tile([3, 512], F32, tag="ruj", name="ruj")
                nc.vector.tensor_scalar(out=ruj[:, :cols], in0=p2[:, :cols],
                                        scalar1=W["dh_b2"][:], scalar2=None,
                                        op0=OP.add)
                nc.sync.dma_start(out=o_raw_u[:, b0 * P:b0 * P + cols],
                                  in_=ruj[:, :cols])
                # u = raw_u * dsg[batch]
                bohj = nb.tile([B, 512], F32, tag="bohj", name="bohj")
                nc.sync.dma_start(out=bohj[:, :cols],
                                  in_=bonehot[:, b0 * P:b0 * P + cols])
                pdn = ps.tile([3, 512], F32, space="PSUM", tag="pseg3",
                              name="pdn")
                nc.tensor.matmul(pdn[:, :cols], lhsT=dsg3[:], rhs=bohj[:, :cols],
                                 start=True, stop=True)
                uj = nb.tile([3, 512], F32, tag="uj", name="uj")
                nc.vector.tensor_tensor(out=uj[:, :cols], in0=ruj[:, :cols],
                                        in1=pdn[:, :cols], op=OP.mult)
                nc.sync.dma_start(out=o_u[:, b0 * P:b0 * P + cols],
                                  in_=uj[:, :cols])
                # stress head
                p3 = ps.tile([64, 512], F32, space="PSUM", tag="pseg0",
                             name="hd_p3")
                nc.tensor.matmul(p3[:, :cols], lhsT=W["sh_w1"][:],
                                 rhs=fmb[:, :cols], start=True, stop=True)
                ts_ = nb.tile([64, 512], F32, tag="t512a", name="hd_ts")
                nc.scalar.activation(out=ts_[:, :cols], in_=p3[:, :cols],
                                     func=AF.Relu, bias=W["sh_b1"][:])
                p4 = ps.tile([1, 512], F32, space="PSUM", tag="pseg1",
                             name="hd_p4")
                nc.tensor.matmul(p4[:, :cols], lhsT=W["sh_w2"][:], rhs=ts_[:, :cols],
                                 start=True, stop=True)
                lsj = nb.tile([1, 512], F32, tag="lsj", name="hd_lsj")
                nc.vector.tensor_scalar(out=lsj[:, :cols], in0=p4[:, :cols],
                                        scalar1=W["sh_b2"][:], scalar2=None,
                                        op0=OP.add)
                nc.vector.tensor_scalar(out=lsj[:, :cols], in0=lsj[:, :cols],
                                        scalar1=0.0, scalar2=30.0,
                                        op0=OP.max, op1=OP.min)
                nc.sync.dma_start(out=o_log_s[:, b0 * P:b0 * P + cols],
                                  in_=lsj[:, :cols])
                sj = nb.tile([1, 512], F32, tag="sj", name="sj")
                nc.scalar.activation(out=sj[:, :cols], in_=lsj[:, :cols],
                                     func=AF.Exp)
                nc.sync.dma_start(out=o_s[:, b0 * P:b0 * P + cols],
                                  in_=sj[:, :cols])
                fj = nb.tile([1, 512], F32, tag="fj", name="fj")
                nc.vector.tensor_scalar(out=fj[:, :cols], in0=sj[:, :cols],
                                        scalar1=1e-8, scalar2=None, op0=OP.add)
                nc.vector.reciprocal(out=fj[:, :cols], in_=fj[:, :cols])
                nc.vector.tensor_scalar(out=fj[:, :cols], in0=fj[:, :cols],
                                        scalar1=YIELD, scalar2=None, op0=OP.mult)
                nc.sync.dma_start(out=o_safety[:, b0 * P:b0 * P + cols],
                                  in_=fj[:, :cols])

    nc.compile()
    return nc


def kernel(**inputs):
    sched, in_maps = _prep(inputs)
    nc = build(sched)
    trace = os.environ.get("GNN_TRACE") == "1"
    kw = {}
    if trace:
        sys.path.insert(0, os.path.dirname(os.path.abspath(__file__)))
        try:
            import profhook
            profhook.install()
            import tempfile
            kw = dict(trace=True, tmpdir=tempfile.mkdtemp(prefix="gnnprof_"))
        except Exception as ex:
            print("profhook unavailable:", ex)
            trace = False
    res = run_bass_kernel_spmd(nc, in_maps, list(range(NCORES)), **kw)
    if trace:
        print(f"HW exec time: {res.exec_time_ns} ns")

    n_real_per = [max(0, min(NC_NODES, N_REAL - c * NC_NODES))
                  for c in range(NCORES)]

    def gather_fm(name):
        parts = []
        for c in range(NCORES):
            a = res.results[c][name]
            parts.append(a[:, :n_real_per[c]].T)
        return np.ascontiguousarray(np.concatenate(parts, 0), dtype=np.float32)

    u = gather_fm("o_u")
    raw_u = gather_fm("o_raw_u")
    s = gather_fm("o_s")
    log_s = gather_fm("o_log_s")
    safety = gather_fm("o_safety")
    dsg = np.ascontiguousarray(res.results[0]["o_dsg"].T, dtype=np.float32)
    dsg_mean = np.float32(res.results[0]["o_dsg_mean"][0, 0])
    return (u, raw_u, s, log_s, dsg_mean, dsg, safety)


# revision 19
# speedup vs baseline: 1.2956x; 1.2956x over previous
"""EngineeringGNN Trainium2 kernel: 8-core SPMD GINE message passing.

Sharding: nodes padded to 100352 and split contiguously over 8 cores
(12544 each). Edges assigned to their destination node's owner, grouped by
128-node destination windows so segment-sum becomes one-hot matmuls
accumulating in PSUM. h[src] rows come from a replicated (AllGathered)
token-major table via indirect DMA gather. Small weights replicated.
"""
import os
import sys
import numpy as np
import ml_dtypes

sys.path.insert(0, "/opt/trn_rl_repo")

import concourse.bass as bass
import concourse.bacc as bacc
import concourse.tile as tile
from concourse import mybir
from concourse.bass_utils import run_bass_kernel_spmd
from concourse.masks import make_identity

F32 = mybir.dt.float32
BF16 = mybir.dt.bfloat16
I32 = mybir.dt.int32
AX = mybir.AxisListType.X
OP = mybir.AluOpType
AF = mybir.ActivationFunctionType

P = 128
H = 128
NCORES = 8
N_REAL = 100000
NPAD = 100352            # 8 * 12544
NC_NODES = NPAD // NCORES  # 12544
NW = NC_NODES // P         # 98 windows per core
B = 16
ND, ED, L = 12, 6, 3
MIN_SCALE = 1e-5
YIELD = 2.5e8
LN_EPS = 1e-5
GG = 8                   # edge tiles per gather group
WBATCH = 8              # windows per node-phase batch (98 = 7*14)
BIG = 1e30


def _prep(inputs):
    """Host-side sharding prep. Returns (schedule, in_maps)."""
    x = np.asarray(inputs["x"], np.float32)
    ea = np.asarray(inputs["edge_attr"], np.float32)
    ei = np.asarray(inputs["edge_index"]).astype(np.int64)
    batch = np.asarray(inputs["batch"]).astype(np.int64)
    pos = np.asarray(inputs["pos"], np.float32)
    src_g, dst_g = ei[0], ei[1]

    owner = dst_g // NC_NODES
    win_g = (dst_g % NC_NODES) // P

    counts = np.zeros((NCORES, NW), np.int64)
    np.add.at(counts, (owner, win_g), 1)
    tiles_per_w = np.maximum(1, np.ceil(counts / P).astype(np.int64)).max(axis=0)
    T = int(tiles_per_w.sum())

    sched_w = np.empty(T, np.int32)
    sched_start = np.zeros(T, bool)
    sched_stop = np.zeros(T, bool)
    t0 = 0
    for w in range(NW):
        tw = int(tiles_per_w[w])
        sched_w[t0:t0 + tw] = w
        sched_start[t0] = True
        sched_stop[t0 + tw - 1] = True
        t0 += tw
    w_tile0 = np.concatenate([[0], np.cumsum(tiles_per_w)[:-1]])

    order = np.lexsort((win_g, owner))
    so, wo = src_g[order], win_g[order]
    do = dst_g[order] % P
    eao = ea[order]
    owner_sorted = owner[order]

    in_maps = []
    z = pos[:, 2]
    for c in range(NCORES):
        lo = np.searchsorted(owner_sorted, c)
        hi = np.searchsorted(owner_sorted, c + 1)
        sc, wc, dc, eac = so[lo:hi], wo[lo:hi], do[lo:hi], eao[lo:hi]
        src_t = np.zeros((P, T), np.int32)
        dst_t = np.full((P, T), -1.0, np.float32)
        ea_t = np.zeros((P, T, ED), np.float32)
        wstart = np.searchsorted(wc, np.arange(NW + 1))
        for w in range(NW):
            a, b = int(wstart[w]), int(wstart[w + 1])
            n = b - a
            t_base = int(w_tile0[w])
            nt = int(np.ceil(n / P)) if n else 0
            for k in range(nt):
                s = a + k * P
                e = min(a + (k + 1) * P, b)
                m = e - s
                src_t[:m, t_base + k] = sc[s:e]
                dst_t[:m, t_base + k] = dc[s:e].astype(np.float32)
                ea_t[:m, t_base + k] = eac[s:e]
        ea_fm = ea_t.transpose(2, 1, 0).reshape(ED, T * P).copy()

        n0 = c * NC_NODES
        n_real = max(0, min(NC_NODES, N_REAL - n0))
        xc = np.zeros((NC_NODES, ND), np.float32)
        xc[:n_real] = x[n0:n0 + n_real]
        bc = np.full(NC_NODES, B, np.int64)
        bc[:n_real] = batch[n0:n0 + n_real]
        zc = np.zeros(NC_NODES, np.float32)
        zc[:n_real] = z[n0:n0 + n_real]

        x_fm = xc.T.copy()
        batch_t = bc.reshape(NW, P).T.astype(np.float32).copy()
        z_t = zc.reshape(NW, P).T.copy()
        boh = np.zeros((B, NC_NODES), np.float32)
        valid = bc < B
        boh[bc[valid], np.nonzero(valid)[0]] = 1.0

        in_maps.append({
            "x_fm": x_fm.astype(ml_dtypes.bfloat16),
            "ea_fm": ea_fm.astype(ml_dtypes.bfloat16),
            "src_i": src_t, "dstrel": dst_t,
            "batch_t": batch_t, "z_t": z_t, "bonehot": boh,
        })

    rep = {
        "fv": np.asarray(inputs["force_vector"], np.float32),
        "mp": np.asarray(inputs["material_params"], np.float32),
        "log_base": np.asarray(inputs["log_base"], np.float32).reshape(1, 1),
    }
    for nm in ["dh_w1", "dh_w2", "sm_w1", "sm_w2", "sh_w1", "sh_w2"]:
        rep[nm] = np.asarray(inputs[nm], np.float32)
    for nm in ["ne_w1", "ne_w2", "ee_w1", "ee_w2"]:
        rep[nm] = np.asarray(inputs[nm], np.float32).astype(ml_dtypes.bfloat16)
    for nm in ["ne_b1", "ne_b2", "ee_b1", "ee_b2", "dh_b1", "dh_b2",
               "sm_b1", "sm_b2", "sh_b1", "sh_b2"]:
        rep[nm] = np.asarray(inputs[nm], np.float32).reshape(-1, 1)
    for nm, key in [("ne_gr", "ne_g"), ("ne_br", "ne_be"),
                    ("ee_gr", "ee_g"), ("ee_br", "ee_be")]:
        rep[nm] = np.tile(np.asarray(inputs[key], np.float32)[None, :], (P, 1))
    cw1 = np.asarray(inputs["conv_w1"], np.float32)
    cw2 = np.asarray(inputs["conv_w2"], np.float32)
    cb1 = np.asarray(inputs["conv_b1"], np.float32)
    cb2 = np.asarray(inputs["conv_b2"], np.float32)
    png = np.asarray(inputs["pn_g"], np.float32)
    pnb = np.asarray(inputs["pn_b"], np.float32)
    for i in range(L):
        rep[f"cw1_{i}"] = cw1[i].astype(ml_dtypes.bfloat16)
        rep[f"cw2_{i}"] = cw2[i].astype(ml_dtypes.bfloat16)
        rep[f"cb1_{i}"] = cb1[i].reshape(-1, 1)
        rep[f"cb2_{i}"] = cb2[i].reshape(-1, 1)
        rep[f"pn_gr_{i}"] = np.tile(png[i][None, :], (P, 1))
        rep[f"pn_br_{i}"] = np.tile(pnb[i][None, :], (P, 1))
    for m in in_maps:
        m.update(rep)
    sched = (T, sched_w, sched_start, sched_stop)
    return sched, in_maps


def _ln_tokmajor(nc, pool, y, g_rep, b_rep, out_ap, eps_ap=None):
    """LayerNorm over features on token-major y [128, 128] -> out_ap."""
    stats = pool.tile([P, 6], F32, tag="ln_stats", name="ln_stats")
    nc.vector.bn_stats(out=stats[:], in_=y[:])
    mv = pool.tile([P, 2], F32, tag="ln_mv", name="ln_mv")
    nc.vector.bn_aggr(out=mv[:], in_=stats[:])
    rstd = pool.tile([P, 1], F32, tag="ln_rstd", name="ln_rstd")
    nc.scalar.activation(out=rstd[:], in_=mv[:, 1:2], func=AF.Sqrt, bias=eps_ap)
    nc.vector.reciprocal(out=rstd[:], in_=rstd[:])
    yn = pool.tile([P, H], F32, tag="ln_yn", name="ln_yn")
    nc.vector.tensor_scalar(out=yn[:], in0=y[:], scalar1=mv[:, 0:1],
                            scalar2=rstd[:], op0=OP.subtract, op1=OP.mult)
    nc.vector.tensor_tensor(out=yn[:], in0=yn[:], in1=g_rep[:], op=OP.mult)
    nc.vector.tensor_tensor(out=out_ap, in0=yn[:], in1=b_rep[:], op=OP.add)


def build(sched):
    T, sched_w, sched_start, sched_stop = sched
    nc = bacc.Bacc("TRN2", target_bir_lowering=False, debug=False,
                   num_devices=NCORES)

    def din(name, shape, dt=F32):
        return nc.dram_tensor(name, list(shape), dt, kind="ExternalInput")

    x_fm = din("x_fm", [ND, NC_NODES], BF16)
    ea_fm = din("ea_fm", [ED, T * P], BF16)
    src_i = din("src_i", [P, T], I32)
    dstrel = din("dstrel", [P, T])
    batch_t = din("batch_t", [P, NW])
    z_t = din("z_t", [P, NW])
    bonehot = din("bonehot", [B, NC_NODES])
    fv = din("fv", [B, 3])
    mp = din("mp", [B, 2])
    log_base = din("log_base", [1, 1])
    wts = {}
    for nm, shp in [("dh_w1", (H, 64)), ("dh_w2", (64, 3)),
                    ("sm_w1", (4, 64)), ("sm_w2", (64, 1)),
                    ("sh_w1", (H, 64)), ("sh_w2", (64, 1)),
                    ("ne_b1", (H, 1)), ("ne_b2", (H, 1)),
                    ("ee_b1", (H, 1)), ("ee_b2", (H, 1)),
                    ("dh_b1", (64, 1)), ("dh_b2", (3, 1)),
                    ("sm_b1", (64, 1)), ("sm_b2", (1, 1)),
                    ("sh_b1", (64, 1)), ("sh_b2", (1, 1)),
                    ("ne_gr", (P, H)), ("ne_br", (P, H)),
                    ("ee_gr", (P, H)), ("ee_br", (P, H))]:
        wts[nm] = din(nm, shp)
    bf_wts = {}
    for nm, shp in [("ne_w1", (ND, H)), ("ne_w2", (H, H)),
                    ("ee_w1", (ED, H)), ("ee_w2", (H, H))]:
        bf_wts[nm] = nc.dram_tensor(nm, list(shp), BF16, kind="ExternalInput")
    for i in range(L):
        for nm, shp in [(f"cb1_{i}", (H, 1)), (f"cb2_{i}", (H, 1)),
                        (f"pn_gr_{i}", (P, H)), (f"pn_br_{i}", (P, H))]:
            wts[nm] = din(nm, shp)
        for nm, shp in [(f"cw1_{i}", (H, H)), (f"cw2_{i}", (H, H))]:
            bf_wts[nm] = nc.dram_tensor(nm, list(shp), BF16, kind="ExternalInput")

    def dout(name, shape):
        return nc.dram_tensor(name, list(shape), F32, kind="ExternalOutput")

    o_raw_u = dout("o_raw_u", [3, NC_NODES])
    o_u = dout("o_u", [3, NC_NODES])
    o_log_s = dout("o_log_s", [1, NC_NODES])
    o_s = dout("o_s", [1, NC_NODES])
    o_safety = dout("o_safety", [1, NC_NODES])
    o_dsg = dout("o_dsg", [1, B])
    o_dsg_mean = dout("o_dsg_mean", [1, 1])

    e_perm = nc.dram_tensor("e_perm", [P, T * H], BF16)
    h_next = [nc.dram_tensor(f"h_next_{i}", [NC_NODES, H], BF16) for i in range(L)]
    h_tbl = [nc.dram_tensor(f"h_tbl_{i}", [NPAD, H], BF16, addr_space="Shared")
             for i in range(L)]
    zst_in = nc.dram_tensor("zst_in", [64, 1], F32)
    zst_out = nc.dram_tensor("zst_out", [64, 1], F32, addr_space="Shared")

    RG = [list(range(NCORES))]

    with tile.TileContext(nc) as tc:
        import contextlib
        with contextlib.ExitStack() as ctx:
            cstp = ctx.enter_context(tc.tile_pool(name="consts", bufs=1))
            hop = ctx.enter_context(tc.tile_pool(name="hown", bufs=1))
            agp = ctx.enter_context(tc.tile_pool(name="aggrp", bufs=1))
            sb = ctx.enter_context(tc.tile_pool(name="work", bufs=2))
            nb = ctx.enter_context(tc.tile_pool(name="nodework", bufs=2))
            lnp = ctx.enter_context(tc.tile_pool(name="lnwork", bufs=2))
            ps = ctx.enter_context(tc.tile_pool(name="ps", bufs=1, space="PSUM"))

            # ---------- constants ----------
            ident = cstp.tile([P, P], F32)
            make_identity(nc, ident[:])
            ident_bf = cstp.tile([P, P], BF16)
            nc.vector.tensor_copy(out=ident_bf[:], in_=ident[:])
            ceps = cstp.tile([P, 1], F32)
            nc.vector.memset(ceps[:], LN_EPS)
            c12 = cstp.tile([B, 1], F32)
            nc.vector.memset(c12[:], 1e-12)
            c6 = cstp.tile([B, 1], F32)
            nc.vector.memset(c6[:], 1e-6)
            iota_f = cstp.tile([P, GG * P], F32)
            nc.gpsimd.iota(iota_f[:], pattern=[[0, GG], [1, P]], base=0,
                           channel_multiplier=0,
                           allow_small_or_imprecise_dtypes=True)
            srcs = cstp.tile([P, T], I32)
            nc.sync.dma_start(out=srcs[:], in_=src_i[:, :])
            dsts = cstp.tile([P, T], F32)
            nc.sync.dma_start(out=dsts[:], in_=dstrel[:, :])
            W = {}
            for nm, hdl in wts.items():
                shp = list(hdl.shape)
                tl = cstp.tile(shp, F32, name=f"w_{nm}")
                nc.sync.dma_start(out=tl[:], in_=hdl[:, :])
                W[nm] = tl
            for nm, hdl in bf_wts.items():
                shp = list(hdl.shape)
                tl = cstp.tile(shp, BF16, name=f"wb_{nm}")
                nc.sync.dma_start(out=tl[:], in_=hdl[:, :])
                W[nm] = tl

            h_own = hop.tile([P, NW, H], F32)
            aggr_sb = agp.tile([P, NW, H], F32)

            def fm_mlp_to(fm_out_cb, rx_hdl, rx_p, w1, b1, w2, b2, g_rep, b_rep,
                          n_tiles, tagpfx):
                """Encoder: feature-major 2-layer MLP + transpose + LN.
                fm_out_cb(j_tile, token_major_ln_out_writer)"""
                for g0 in range(0, n_tiles, 4):
                    gn = min(4, n_tiles - g0)
                    cols = gn * P
                    rx = nb.tile([rx_p, 4 * P], BF16, tag="rx", name="rx")
                    nc.sync.dma_start(out=rx[:, :cols],
                                      in_=rx_hdl[:, g0 * P:g0 * P + cols])
                    p1 = ps.tile([P, 512], F32, space="PSUM", tag="pb", name="p1")
                    nc.tensor.matmul(p1[:, :cols], lhsT=w1[:], rhs=rx[:, :cols],
                                     start=True, stop=True)
                    t1 = nb.tile([P, 4 * P], BF16, tag="t512a1", name="t1")
                    nc.scalar.activation(out=t1[:, :cols], in_=p1[:, :cols],
                                         func=AF.Relu, bias=b1[:])
                    p2 = ps.tile([P, 512], F32, space="PSUM", tag="pc", name="p2")
                    nc.tensor.matmul(p2[:, :cols], lhsT=w2[:], rhs=t1[:, :cols],
                                     start=True, stop=True)
                    t2 = nb.tile([P, 4 * P], F32, tag="t512b", name="t2")
                    nc.vector.tensor_scalar(out=t2[:, :cols], in0=p2[:, :cols],
                                            scalar1=b2[:], scalar2=None,
                                            op0=OP.add)
                    for j in range(gn):
                        pt = ps.tile([P, P], F32, space="PSUM", tag="pa",
                                     name="ptr", bufs=2)
                        nc.tensor.transpose(out=pt[:],
                                            in_=t2[:, j * P:(j + 1) * P],
                                            identity=ident[:])
                        yj = nb.tile([P, P], F32, tag="t128a", name="yj")
                        nc.scalar.copy(out=yj[:], in_=pt[:])
                        fm_out_cb(g0 + j, yj, g_rep, b_rep)

            # ---------- node encoder ----------
            def ne_out(w, yj, g_rep, b_rep):
                _ln_tokmajor(nc, lnp, yj, g_rep, b_rep, h_own[:, w, :], ceps[:])
                hb = nb.tile([P, P], BF16, tag="hb16", name="hb16")
                nc.vector.tensor_copy(out=hb[:], in_=h_own[:, w, :])
                nc.sync.dma_start(out=h_next[0][w * P:(w + 1) * P, :], in_=hb[:])
            fm_mlp_to(ne_out, x_fm, ND, W["ne_w1"], W["ne_b1"], W["ne_w2"],
                      W["ne_b2"], W["ne_gr"], W["ne_br"], NW, "ne")

            # ---------- edge encoder ----------
            def ee_out(t, yj, g_rep, b_rep):
                ej = nb.tile([P, P], BF16, tag="t128b", name="ej")
                _ln_tokmajor(nc, lnp, yj, g_rep, b_rep, ej[:], ceps[:])
                nc.sync.dma_start(out=e_perm[:, t * H:(t + 1) * H], in_=ej[:])
            fm_mlp_to(ee_out, ea_fm, ED, W["ee_w1"], W["ee_b1"], W["ee_w2"],
                      W["ee_b2"], W["ee_gr"], W["ee_br"], T, "ee")

            nc.gpsimd.collective_compute(
                "AllGather", OP.bypass, replica_groups=RG,
                ins=[h_next[0][:, :]], outs=[h_tbl[0][:, :]])

            # ---------- GINE layers ----------
            for li in range(L):
                tbl = h_tbl[li]
                pseg = {}
                for g0 in range(0, T, GG):
                    gn = min(GG, T - g0)
                    cols = gn * P
                    hg = sb.tile([P, GG * P], BF16, tag="hg", name="hg")
                    for k in range(gn):
                        t = g0 + k
                        nc.gpsimd.indirect_dma_start(
                            out=hg[:, k * H:(k + 1) * H], out_offset=None,
                            in_=tbl[:, :],
                            in_offset=bass.IndirectOffsetOnAxis(
                                ap=srcs[:, t:t + 1], axis=0))
                    eg = sb.tile([P, GG * P], BF16, tag="eg", name="eg")
                    nc.sync.dma_start(out=eg[:, :cols],
                                      in_=e_perm[:, g0 * H:g0 * H + cols])
                    mg = sb.tile([P, GG * P], BF16, tag="mg", name="mg")
                    nc.vector.tensor_add(out=mg[:, :cols], in0=hg[:, :cols],
                                         in1=eg[:, :cols])
                    nc.scalar.activation(out=mg[:, :cols], in_=mg[:, :cols],
                                         func=AF.Relu)
                    sg = sb.tile([P, GG * P], BF16, tag="sg", name="sg")
                    d_ap = bass.AP(tensor=dsts.tensor,
                                   offset=dsts[:, g0:g0 + gn].offset,
                                   ap=[dsts.ap[0], [dsts.ap[1][0], gn], [0, P]])
                    nc.vector.tensor_tensor(out=sg[:, :cols], in0=d_ap,
                                            in1=iota_f[:, :cols], op=OP.is_equal)
                    for k in range(gn):
                        t = g0 + k
                        w = int(sched_w[t])
                        if bool(sched_start[t]):
                            pseg[w] = ps.tile([P, H], F32, space="PSUM",
                                              tag=f"pseg{w % 4}", name=f"psg{w % 4}")
                        nc.tensor.matmul(pseg[w][:],
                                         lhsT=sg[:, k * P:(k + 1) * P],
                                         rhs=mg[:, k * H:(k + 1) * H],
                                         start=bool(sched_start[t]),
                                         stop=bool(sched_stop[t]))
                        if bool(sched_stop[t]):
                            nc.vector.tensor_copy(out=aggr_sb[:, w, :],
                                                  in_=pseg[w][:])
                            del pseg[w]

                for b0 in range(0, NW, WBATCH):
                    bn_ = min(WBATCH, NW - b0)
                    fmb = nb.tile([P, WBATCH * P], BF16, tag="fmb", name="fmb",
                                  bufs=1)
                    for j in range(bn_):
                        w = b0 + j
                        ain = nb.tile([P, P], F32, tag="t128a", name="ain")
                        nc.vector.tensor_add(out=ain[:], in0=h_own[:, w, :],
                                             in1=aggr_sb[:, w, :])
                        pt = ps.tile([P, P], F32, space="PSUM", tag="pa",
                                     name="np_pt", bufs=2)
                        nc.tensor.transpose(out=pt[:], in_=ain[:],
                                            identity=ident[:])
                        nc.scalar.copy(out=fmb[:, j * P:(j + 1) * P], in_=pt[:])
                    for s0 in range(0, bn_ * P, 512):
                        sc = min(512, bn_ * P - s0)
                        p1 = ps.tile([P, 512], F32, space="PSUM", tag="pb",
                                     name="np_p1")
                        nc.tensor.matmul(p1[:, :sc], lhsT=W[f"cw1_{li}"][:],
                                         rhs=fmb[:, s0:s0 + sc], start=True,
                                         stop=True)
                        tm = nb.tile([P, 512], BF16, tag="t512a1", name="np_tm")
                        nc.scalar.activation(out=tm[:, :sc], in_=p1[:, :sc],
                                             func=AF.Relu, bias=W[f"cb1_{li}"][:])
                        p2 = ps.tile([P, 512], F32, space="PSUM", tag="pc",
                                     name="np_p2")
                        nc.tensor.matmul(p2[:, :sc], lhsT=W[f"cw2_{li}"][:],
                                         rhs=tm[:, :sc], start=True, stop=True)
                        nc.scalar.activation(out=fmb[:, s0:s0 + sc],
                                             in_=p2[:, :sc], func=AF.Relu,
                                             bias=W[f"cb2_{li}"][:])
                    for j in range(bn_):
                        w = b0 + j
                        pt = ps.tile([P, P], BF16, space="PSUM", tag="pa",
                                     name="np_pt2", bufs=2)
                        nc.tensor.transpose(out=pt[:], in_=fmb[:, j * P:(j + 1) * P],
                                            identity=ident_bf[:])
                        yj = nb.tile([P, P], F32, tag="t128a", name="np_yj")
                        nc.vector.tensor_add(out=yj[:], in0=h_own[:, w, :],
                                             in1=pt[:])
                        _ln_tokmajor(nc, lnp, yj, W[f"pn_gr_{li}"],
                                     W[f"pn_br_{li}"], h_own[:, w, :], ceps[:])
                        if li < L - 1:
                            hb = nb.tile([P, P], BF16, tag="hb16", name="hb16")
                            nc.vector.tensor_copy(out=hb[:], in_=h_own[:, w, :])
                            nc.sync.dma_start(
                                out=h_next[li + 1][w * P:(w + 1) * P, :],
                                in_=hb[:])
                if li < L - 1:
                    nc.gpsimd.collective_compute(
                        "AllGather", OP.bypass, replica_groups=RG,
                        ins=[h_next[li + 1][:, :]], outs=[h_tbl[li + 1][:, :]])

            # ---------- per-graph scale path (independent of h) ----------
            bt_sb = cstp.tile([P, NW], F32)
            nc.sync.dma_start(out=bt_sb[:], in_=batch_t[:, :])
            zt_sb = cstp.tile([P, NW], F32)
            nc.sync.dma_start(out=zt_sb[:], in_=z_t[:, :])
            nz_sb = cstp.tile([P, NW], F32)
            nc.vector.tensor_scalar(out=nz_sb[:], in0=zt_sb[:], scalar1=-1.0,
                                    scalar2=None, op0=OP.mult)
            zred = cstp.tile([P, 64], F32)
            for b in range(B):
                msk = lnp.tile([P, NW], mybir.dt.uint8, tag="zmask", name="zmask")
                nc.vector.tensor_scalar(out=msk[:], in0=bt_sb[:],
                                        scalar1=float(b), scalar2=None,
                                        op0=OP.is_equal)
                zm = lnp.tile([P, NW], F32, tag="zm", name="zm")
                nc.vector.memset(zm[:], -BIG)
                nc.vector.copy_predicated(zm[:], msk[:], zt_sb[:])
                nc.vector.tensor_reduce(out=zred[:, b:b + 1], in_=zm[:],
                                        axis=AX, op=OP.max)
                zm2 = lnp.tile([P, NW], F32, tag="zm", name="zm2")
                nc.vector.memset(zm2[:], -BIG)
                nc.vector.copy_predicated(zm2[:], msk[:], nz_sb[:])
                nc.vector.tensor_reduce(out=zred[:, 32 + b:33 + b], in_=zm2[:],
                                        axis=AX, op=OP.max)
            pzt = ps.tile([64, P], F32, space="PSUM", tag="pa", name="pzt",
                          bufs=2)
            nc.tensor.transpose(out=pzt[:], in_=zred[:], identity=ident[:])
            zcols = cstp.tile([64, P], F32)
            nc.scalar.copy(out=zcols[:], in_=pzt[:])
            zst = cstp.tile([64, 1], F32)
            nc.vector.tensor_reduce(out=zst[:], in_=zcols[:], axis=AX, op=OP.max)
            nc.sync.dma_start(out=zst_in[:, :], in_=zst[:])
            nc.gpsimd.collective_compute(
                "AllReduce", OP.max, replica_groups=RG,
                ins=[zst_in[:, :]], outs=[zst_out[:, :]])
            zmax_t = cstp.tile([B, 1], F32)
            nc.sync.dma_start(out=zmax_t[:], in_=zst_out[0:B, :])
            zmin_t = cstp.tile([B, 1], F32)
            nc.sync.dma_start(out=zmin_t[:], in_=zst_out[32:32 + B, :])
            geom = cstp.tile([B, 1], F32)
            nc.vector.tensor_tensor(out=geom[:], in0=zmax_t[:],
                                    in1=zmin_t[:], op=OP.add)
            nc.vector.tensor_scalar(out=geom[:], in0=geom[:], scalar1=1e-6,
                                    scalar2=None, op0=OP.max)
            feats = cstp.tile([B, 4], F32)
            fv_sb = cstp.tile([B, 3], F32)
            nc.sync.dma_start(out=fv_sb[:], in_=fv[:, :])
            mp_sb = cstp.tile([B, 2], F32)
            nc.sync.dma_start(out=mp_sb[:], in_=mp[:, :])
            fsq = cstp.tile([B, 3], F32)
            nc.vector.tensor_tensor(out=fsq[:], in0=fv_sb[:], in1=fv_sb[:],
                                    op=OP.mult)
            fm_ = cstp.tile([B, 1], F32)
            nc.vector.tensor_reduce(out=fm_[:], in_=fsq[:], axis=AX, op=OP.add)
            nc.scalar.activation(out=fm_[:], in_=fm_[:], func=AF.Sqrt)
            nc.scalar.activation(out=feats[:, 0:1], in_=fm_[:], func=AF.Ln,
                                 bias=1.0)
            em_ = cstp.tile([B, 1], F32)
            nc.vector.tensor_scalar(out=em_[:], in0=mp_sb[:, 0:1], scalar1=1.0,
                                    scalar2=None, op0=OP.max)
            nc.scalar.activation(out=feats[:, 1:2], in_=em_[:], func=AF.Ln,
                                 bias=c12[:])
            nc.vector.tensor_scalar(out=feats[:, 2:3], in0=mp_sb[:, 1:2],
                                    scalar1=0.0, scalar2=0.49, op0=OP.max,
                                    op1=OP.min)
            nc.scalar.activation(out=feats[:, 3:4], in_=geom[:], func=AF.Ln,
                                 bias=c6[:])
            pft = ps.tile([4, B], F32, space="PSUM", tag="pb", name="pft")
            nc.tensor.transpose(out=pft[:], in_=feats[:],
                                identity=ident[0:B, 0:B])
            ftm = cstp.tile([4, B], F32)
            nc.scalar.copy(out=ftm[:], in_=pft[:])
            psm1 = ps.tile([64, B], F32, space="PSUM", tag="pc", name="psm1")
            nc.tensor.matmul(psm1[:], lhsT=W["sm_w1"][:], rhs=ftm[:], start=True,
                             stop=True)
            tsm = cstp.tile([64, B], F32)
            nc.scalar.activation(out=tsm[:], in_=psm1[:], func=AF.Relu,
                                 bias=W["sm_b1"][:])
            psm2 = ps.tile([1, B], F32, space="PSUM", tag="pseg2", name="psm2")
            nc.tensor.matmul(psm2[:], lhsT=W["sm_w2"][:], rhs=tsm[:], start=True,
                             stop=True)
            lmu = cstp.tile([1, B], F32)
            nc.vector.tensor_scalar(out=lmu[:], in0=psm2[:],
                                    scalar1=W["sm_b2"][:], scalar2=None,
                                    op0=OP.add)
            nc.vector.tensor_scalar(out=lmu[:], in0=lmu[:], scalar1=-8.0,
                                    scalar2=8.0, op0=OP.max, op1=OP.min)
            nc.scalar.activation(out=lmu[:], in_=lmu[:], func=AF.Exp)
            lb_sb = cstp.tile([1, 1], F32)
            nc.sync.dma_start(out=lb_sb[:], in_=log_base[:, :])
            base_sb = cstp.tile([1, 1], F32)
            nc.scalar.activation(out=base_sb[:], in_=lb_sb[:], func=AF.Exp)
            nc.scalar.activation(out=base_sb[:], in_=base_sb[:], func=AF.Ln,
                                 bias=1.0)
            nc.vector.tensor_scalar(out=base_sb[:], in0=base_sb[:],
                                    scalar1=MIN_SCALE, scalar2=None, op0=OP.add)
            dsg_sb = cstp.tile([1, B], F32)
            nc.vector.tensor_scalar(out=dsg_sb[:], in0=lmu[:], scalar1=base_sb[:],
                                    scalar2=MIN_SCALE, op0=OP.mult, op1=OP.max)
            nc.sync.dma_start(out=o_dsg[:, :], in_=dsg_sb[:])
            dmean = cstp.tile([1, 1], F32)
            nc.vector.tensor_reduce(out=dmean[:], in_=dsg_sb[:], axis=AX,
                                    op=OP.add)
            nc.vector.tensor_scalar(out=dmean[:], in0=dmean[:],
                                    scalar1=1.0 / B, scalar2=None, op0=OP.mult)
            nc.sync.dma_start(out=o_dsg_mean[:, :], in_=dmean[:])
            pdt = ps.tile([B, 1], F32, space="PSUM", tag="pseg3", name="pdt")
            nc.tensor.transpose(out=pdt[:], in_=dsg_sb[:],
                                identity=ident[0:1, 0:1])
            dsg_t = cstp.tile([B, 1], F32)
            nc.scalar.copy(out=dsg_t[:], in_=pdt[:])
            dsg3 = cstp.tile([B, 3], F32)
            nc.vector.tensor_copy(out=dsg3[:], in_=dsg_t[:].to_broadcast([B, 3]))

            # ---------- heads (streamed per 4-window group) ----------
            for b0 in range(0, NW, 4):
                bn_ = min(4, NW - b0)
                cols = bn_ * P
                fmb = nb.tile([P, 512], F32, tag="t512b", name="hd_fmb")
                for j in range(bn_):
                    pt = ps.tile([P, P], F32, space="PSUM", tag="pa",
                                 name="hd_pt", bufs=2)
                    nc.tensor.transpose(out=pt[:], in_=h_own[:, b0 + j, :],
                                        identity=ident[:])
                    nc.scalar.copy(out=fmb[:, j * P:(j + 1) * P], in_=pt[:])
                # displacement head
                p1 = ps.tile([64, 512], F32, space="PSUM", tag="pb", name="hd_p1")
                nc.tensor.matmul(p1[:, :cols], lhsT=W["dh_w1"][:],
                                 rhs=fmb[:, :cols], start=True, stop=True)
                td = nb.tile([64, 512], F32, tag="t512a", name="hd_td")
                nc.scalar.activation(out=td[:, :cols], in_=p1[:, :cols],
                                     func=AF.Relu, bias=W["dh_b1"][:])
                p2 = ps.tile([3, 512], F32, space="PSUM", tag="pc", name="hd_p2")
                nc.tensor.matmul(p2[:, :cols], lhsT=W["dh_w2"][:], rhs=td[:, :cols],
                                 start=True, stop=True)
                ruj = nb.tile([3, 512], F32, tag="ruj", name="ruj")
                nc.vector.tensor_scalar(out=ruj[:, :cols], in0=p2[:, :cols],
                                        scalar1=W["dh_b2"][:], scalar2=None,
                                        op0=OP.add)
                nc.sync.dma_start(out=o_raw_u[:, b0 * P:b0 * P + cols],
                                  in_=ruj[:, :cols])
                # u = raw_u * dsg[batch]
                bohj = nb.tile([B, 512], F32, tag="bohj", name="bohj")
                nc.sync.dma_start(out=bohj[:, :cols],
                                  in_=bonehot[:, b0 * P:b0 * P + cols])
                pdn = ps.tile([3, 512], F32, space="PSUM", tag="pseg3",
                              name="pdn")
                nc.tensor.matmul(pdn[:, :cols], lhsT=dsg3[:], rhs=bohj[:, :cols],
                                 start=True, stop=True)
                uj = nb.tile([3, 512], F32, tag="uj", name="uj")
                nc.vector.tensor_tensor(out=uj[:, :cols], in0=ruj[:, :cols],
                                        in1=pdn[:, :cols], op=OP.mult)
                nc.sync.dma_start(out=o_u[:, b0 * P:b0 * P + cols],
                                  in_=uj[:, :cols])
                # stress head
                p3 = ps.tile([64, 512], F32, space="PSUM", tag="pseg0",
                             name="hd_p3")
                nc.tensor.matmul(p3[:, :cols], lhsT=W["sh_w1"][:],
                                 rhs=fmb[:, :cols], start=True, stop=True)
                ts_ = nb.tile([64, 512], F32, tag="t512a", name="hd_ts")
                nc.scalar.activation(out=ts_[:, :cols], in_=p3[:, :cols],
                                     func=AF.Relu, bias=W["sh_b1"][:])
                p4 = ps.tile([1, 512], F32, space="PSUM", tag="pseg1",
                             name="hd_p4")
                nc.tensor.matmul(p4[:, :cols], lhsT=W["sh_w2"][:], rhs=ts_[:, :cols],
                                 start=True, stop=True)
                lsj = nb.tile([1, 512], F32, tag="lsj", name="hd_lsj")
                nc.vector.tensor_scalar(out=lsj[:, :cols], in0=p4[:, :cols],
                                        scalar1=W["sh_b2"][:], scalar2=None,
                                        op0=OP.add)
                nc.vector.tensor_scalar(out=lsj[:, :cols], in0=lsj[:, :cols],
                                        scalar1=0.0, scalar2=30.0,
                                        op0=OP.max, op1=OP.min)
                nc.sync.dma_start(out=o_log_s[:, b0 * P:b0 * P + cols],
                                  in_=lsj[:, :cols])
                sj = nb.tile([1, 512], F32, tag="sj", name="sj")
                nc.scalar.activation(out=sj[:, :cols], in_=lsj[:, :cols],
                                     func=AF.Exp)
                nc.sync.dma_start(out=o_s[:, b0 * P:b0 * P + cols],
                                  in_=sj[:, :cols])
                fj = nb.tile([1, 512], F32, tag="fj", name="fj")
                nc.vector.tensor_scalar(out=fj[:, :cols], in0=sj[:, :cols],
                                        scalar1=1e-8, scalar2=None, op0=OP.add)
                nc.vector.reciprocal(out=fj[:, :cols], in_=fj[:, :cols])
                nc.vector.tensor_scalar(out=fj[:, :cols], in0=fj[:, :cols],
                                        scalar1=YIELD, scalar2=None, op0=OP.mult)
                nc.sync.dma_start(out=o_safety[:, b0 * P:b0 * P + cols],
                                  in_=fj[:, :cols])

    nc.compile()
    return nc


def kernel(**inputs):
    sched, in_maps = _prep(inputs)
    nc = build(sched)
    trace = os.environ.get("GNN_TRACE") == "1"
    kw = {}
    if trace:
        sys.path.insert(0, os.path.dirname(os.path.abspath(__file__)))
        try:
            import profhook
            profhook.install()
            import tempfile
            kw = dict(trace=True, tmpdir=tempfile.mkdtemp(prefix="gnnprof_"))
        except Exception as ex:
            print("profhook unavailable:", ex)
            trace = False
    res = run_bass_kernel_spmd(nc, in_maps, list(range(NCORES)), **kw)
    if trace:
        print(f"HW exec time: {res.exec_time_ns} ns")

    n_real_per = [max(0, min(NC_NODES, N_REAL - c * NC_NODES))
                  for c in range(NCORES)]

    def gather_fm(name):
        parts = []
        for c in range(NCORES):
            a = res.results[c][name]
            parts.append(a[:, :n_real_per[c]].T)
        return np.ascontiguousarray(np.concatenate(parts, 0), dtype=np.float32)

    u = gather_fm("o_u")
    raw_u = gather_fm("o_raw_u")
    s = gather_fm("o_s")
    log_s = gather_fm("o_log_s")
    safety = gather_fm("o_safety")
    dsg = np.ascontiguousarray(res.results[0]["o_dsg"].T, dtype=np.float32)
    dsg_mean = np.float32(res.results[0]["o_dsg_mean"][0, 0])
    return (u, raw_u, s, log_s, dsg_mean, dsg, safety)
